# revision 13
# baseline (speedup 1.0000x reference)
"""Trainium2 Bass kernel for the batched differentiable EKF.

Problem: B=8192 independent rows, T=2048 sequential EKF steps per row
(2-dim state Kalman filter, scalar observation). Output [B, T, 2].

Strategy:
- Data parallel: B sharded 1024 rows/core across 8 cores.
- Time parallel within a core: the EKF forgets its initial condition in
  ~48 steps (Riccati contraction + observed position). T is split into C
  chunks of L steps; every chunk is preceded by a W-step warmup from a
  cold init (x=[z,dz], P=I). Chunk 0's "warmup" IS the true filter from
  the exact reference init, so its warmup outputs are kept; other chunks'
  warmup outputs are discarded. W=64 makes warmup error < 1e-6.
- Lanes: 128 partitions x (G row-groups * C chunks) in the free dim.
  Each time step is a handful of fused elementwise ops on [128, G*C].
- Per-step op schedule splits across DVE (most), GPSIMD (adds/STTs), with
  the bulk input derivation (sigmoid etc.) on the Scalar (ACT) engine.
"""

import numpy as np

import concourse.bass as bass
import concourse.bacc as bacc
import concourse.mybir as mybir
import concourse.tile as tile

F32 = mybir.dt.float32
ALU = mybir.AluOpType
PART = 128

# ----------------------------------------------------------------------
# Custom DVE ops (registered at import; sha computed dynamically)
# ----------------------------------------------------------------------
from concourse.dve_spec import Spec, Src0, Src1, One, lower
import concourse.dve_ops as dve_ops_mod
from concourse.dve_ops import DveOp, OPS
from concourse.dve_uop import DveOpSpec


def _register_dve_op(name: str, spec: Spec) -> DveOp:
    for op in OPS:
        if op.name == name:
            return op
    shas = {}
    for ver in ("v3", "v4"):
        uops = lower(spec, ver=ver)
        shas[ver] = DveOpSpec(name=name, opcode=0, uops=uops, rd1_en=True).sha(ver)
    op = DveOp(name, spec, subdim=False, uops_sha=shas)
    OPS.append(op)
    dve_ops_mod.CUSTOM_DVE_SPECS[name] = spec
    dve_ops_mod._SUB_OPCODE_FOR_NAME[name] = (
        dve_ops_mod._CUSTOM_DVE_ROW_BASE + len(OPS) - 1
    )
    assert dve_ops_mod._SUB_OPCODE_FOR_NAME[name] < 0x20
    return op


# out = in0 * (1 - in1)   (posterior covariance: P' = pp * (1 - K0))
OMK = _register_dve_op(
    "EKF_OMK",
    Spec(
        body=Src0 * (One - Src1),
        reference=lambda in0, in1, s0, s1, imm2: (
            in0 * (1.0 - np.asarray(in1).reshape(np.asarray(in0).shape))
        ).astype(np.float32),
    ),
)


# ----------------------------------------------------------------------
# Kernel builder (single core, B_loc rows)
# ----------------------------------------------------------------------
def build_core_kernel(b_loc: int, t_len: int, c_chunks: int, warm: int, slab: int):
    """Build and compile the per-core Bass module.

    Returns the compiled bacc module `nc` with inputs price/hurst/vol
    [b_loc, t_len] f32 and output out [b_loc, t_len, 2] f32.
    """
    G = b_loc // PART
    C = c_chunks
    W = warm
    L = (t_len - W) // C
    assert C * L + W == t_len, (t_len, C, L, W)
    GC = G * C
    steps = W + L
    assert W % slab == 0

    nc = bacc.Bacc("TRN2", target_bir_lowering=False, debug=False)
    pr_h = nc.dram_tensor("price", [b_loc, t_len], F32, kind="ExternalInput")
    hu_h = nc.dram_tensor("hurst", [b_loc, t_len], F32, kind="ExternalInput")
    vs_h = nc.dram_tensor("vol_sigma", [b_loc, t_len], F32, kind="ExternalInput")
    out_h = nc.dram_tensor("out", [b_loc, t_len, 2], F32, kind="ExternalOutput")

    def in_slab_src(handle, g, s0, ns):
        # [p, c, s] <- dram[(g*128+p), c*L + s0 + s]
        return bass.AP(
            tensor=handle,
            offset=g * PART * t_len + s0,
            ap=[[t_len, PART], [L, C], [1, ns]],
        )

    def out_slab_dst(g, s0, ns, all_chunks):
        cdim = [L * 2, C] if all_chunks else [L * 2, 1]
        return bass.AP(
            tensor=out_h,
            offset=g * PART * t_len * 2 + s0 * 2,
            ap=[[t_len * 2, PART], cdim, [1, ns * 2]],
        )

    slabs = []
    s0 = 0
    while s0 < steps:
        ns = min(slab, steps - s0)
        slabs.append((s0, ns))
        s0 += ns

    def dup2(ap2d):
        # [128, GC] -> [128, GC, 2] zero-stride broadcast
        return ap2d.unsqueeze(2).broadcast_to([PART, ap2d.shape[1], 2])

    with tile.TileContext(nc) as tc:
        with (
            tc.tile_pool(name="io", bufs=2) as iop,
            tc.tile_pool(name="ost", bufs=3) as ostp,
            tc.tile_pool(name="st", bufs=4) as stp,
            tc.tile_pool(name="ini", bufs=1) as inip,
        ):
            # ---- initial state tiles ----
            x0_i = inip.tile([PART, GC], F32, tag="x0i")
            x1_i = inip.tile([PART, GC], F32, tag="x1i")
            p01_i = inip.tile([PART, 2 * GC], F32, tag="p01i")
            p11_i = inip.tile([PART, GC], F32, tag="p11i")
            p01_iv = p01_i[:].rearrange("p (gc two) -> p gc two", two=2)
            bias_m5 = inip.tile([PART, 1], F32, tag="bm5")
            nc.gpsimd.memset(bias_m5[:], -5.0)
            nc.gpsimd.memset(p01_i[:], 0.0)
            nc.gpsimd.memset(p11_i[:], 1.0)
            # p00 = 1 on even slots (after the memset-0)
            nc.vector.tensor_scalar_add(p01_iv[:, :, 0], p01_iv[:, :, 0], 1.0)

            # python-side carried refs
            prev = {}

            for si, (s0, ns) in enumerate(slabs):
                z_sl = iop.tile([PART, GC * ns], F32, tag="z")
                h_sl = iop.tile([PART, GC * ns], F32, tag="h")
                v_sl = iop.tile([PART, GC * ns], F32, tag="v")
                a_sl = iop.tile([PART, GC * ns], F32, tag="a")
                scl_sl = iop.tile([PART, GC * ns], F32, tag="scl")
                q_sl = iop.tile([PART, GC * ns], F32, tag="q")
                o_sl = ostp.tile([PART, GC * ns * 2], F32, tag="o")

                for tl, hd in ((z_sl, pr_h), (h_sl, hu_h), (v_sl, vs_h)):
                    v4 = tl[:].rearrange(
                        "p (g c s) -> p g c s", g=G, c=C, s=ns
                    )
                    for g in range(G):
                        nc.sync.dma_start(
                            v4[:, g], in_slab_src(hd, g, s0, ns)
                        )

                # bulk derivation (ACT + GPSIMD)
                # a = 0.5 + 0.5*sigmoid(10h - 5)   (dt=1 so F01 = a = rho)
                nc.scalar.activation(
                    a_sl[:], h_sl[:], mybir.ActivationFunctionType.Sigmoid,
                    bias=bias_m5[:], scale=10.0,
                )
                nc.scalar.activation(
                    a_sl[:], a_sl[:], mybir.ActivationFunctionType.Copy,
                    bias=0.5, scale=0.5,
                )
                # scl = max(100*sig, 1);  q = 0.1*scl
                nc.gpsimd.tensor_scalar(
                    scl_sl[:], v_sl[:], 100.0, 1.0, ALU.mult, ALU.max
                )
                nc.gpsimd.tensor_scalar_mul(q_sl[:], scl_sl[:], 0.1)

                zv = z_sl[:].rearrange("p (gc s) -> p gc s", gc=GC, s=ns)
                av = a_sl[:].rearrange("p (gc s) -> p gc s", gc=GC, s=ns)
                sclv = scl_sl[:].rearrange("p (gc s) -> p gc s", gc=GC, s=ns)
                qv_sl = q_sl[:].rearrange("p (gc s) -> p gc s", gc=GC, s=ns)
                ov = o_sl[:].rearrange("p (gc s two) -> p gc s two", s=ns, two=2)

                if si == 0:
                    # x init from z columns 0/1: x0 = z0, x1 = z1 - z0
                    nc.vector.tensor_copy(x0_i[:], zv[:, :, 0])
                    nc.vector.tensor_tensor(
                        out=x1_i[:], in0=zv[:, :, 1], in1=zv[:, :, 0],
                        op=ALU.subtract,
                    )
                    prev = dict(
                        p01=p01_i, p11=p11_i[:], x0=x0_i[:], x1=x1_i[:]
                    )

                for s in range(ns):
                    A = av[:, :, s]
                    SCL = sclv[:, :, s]
                    Q = qv_sl[:, :, s]
                    Z = zv[:, :, s]
                    p01_prev_v = prev["p01"][:].rearrange(
                        "p (gc two) -> p gc two", two=2
                    )
                    p00p = p01_prev_v[:, :, 0]
                    p01p = p01_prev_v[:, :, 1]
                    p11p = prev["p11"]
                    x0p = prev["x0"]
                    x1p = prev["x1"]

                    pp = stp.tile([PART, 2 * GC], F32, tag="pp")
                    ppv = pp[:].rearrange("p (gc two) -> p gc two", two=2)
                    kk = stp.tile([PART, 2 * GC], F32, tag="kk")
                    kkv = kk[:].rearrange("p (gc two) -> p gc two", two=2)
                    p01_n = stp.tile([PART, 2 * GC], F32, tag="p01")
                    p11_n = stp.tile([PART, GC], F32, tag="p11")
                    t1 = stp.tile([PART, GC], F32, tag="t1")
                    g2 = stp.tile([PART, GC], F32, tag="g2")
                    m = stp.tile([PART, GC], F32, tag="m")
                    pq = stp.tile([PART, GC], F32, tag="pq")
                    sS = stp.tile([PART, GC], F32, tag="sS")
                    rr = stp.tile([PART, GC], F32, tag="rr")
                    pp11 = stp.tile([PART, GC], F32, tag="pp11")
                    t3 = stp.tile([PART, GC], F32, tag="t3")
                    t4 = stp.tile([PART, GC], F32, tag="t4")
                    xp = stp.tile([PART, GC], F32, tag="xp")
                    yy = stp.tile([PART, GC], F32, tag="yy")
                    yk = stp.tile([PART, 2 * GC], F32, tag="yk")
                    ykv = yk[:].rearrange("p (gc two) -> p gc two", two=2)

                    V = nc.vector
                    GP = nc.gpsimd
                    # --- covariance predict ---
                    V.tensor_tensor(out=t1[:], in0=A, in1=p11p, op=ALU.mult)
                    V.tensor_tensor(
                        out=ppv[:, :, 1], in0=p01p, in1=t1[:], op=ALU.add
                    )  # pp01
                    V.scalar_tensor_tensor(
                        out=g2[:], in0=p01p, scalar=2.0, in1=t1[:],
                        op0=ALU.mult, op1=ALU.add,
                    )  # 2*p01 + a*p11
                    V.tensor_tensor(out=m[:], in0=A, in1=g2[:], op=ALU.mult)
                    GP.tensor_tensor(
                        out=pq[:], in0=Q, in1=p00p, op=ALU.add
                    )  # p00 + q
                    V.tensor_tensor(
                        out=ppv[:, :, 0], in0=pq[:], in1=m[:], op=ALU.add
                    )  # pp00
                    # --- gain ---
                    V.scalar_tensor_tensor(
                        out=sS[:], in0=SCL, scalar=1e-6, in1=ppv[:, :, 0],
                        op0=ALU.add, op1=ALU.add,
                    )  # S + 1e-6
                    V.reciprocal_approx_fast(out=rr[:], in_=sS[:])
                    V.tensor_tensor(
                        out=kkv, in0=ppv, in1=dup2(rr[:]), op=ALU.mult
                    )  # K0,K1
                    # --- covariance update ---
                    V._custom_dve(
                        OMK,
                        out=p01_n[:].rearrange("p (gc two) -> p gc two", two=2),
                        in0=ppv,
                        in1=dup2(kkv[:, :, 0]),
                    )  # p00', p01'
                    GP.tensor_tensor(
                        out=pp11[:], in0=Q, in1=p11p, op=ALU.add
                    )  # p11 + q
                    GP.tensor_tensor(
                        out=t3[:], in0=kkv[:, :, 1], in1=ppv[:, :, 1],
                        op=ALU.mult,
                    )
                    GP.tensor_tensor(
                        out=p11_n[:], in0=pp11[:], in1=t3[:], op=ALU.subtract
                    )
                    # --- state update ---
                    V.tensor_tensor(out=t4[:], in0=A, in1=x1p, op=ALU.mult)
                    V.tensor_tensor(out=xp[:], in0=x0p, in1=t4[:], op=ALU.add)
                    V.tensor_tensor(out=yy[:], in0=Z, in1=xp[:], op=ALU.subtract)
                    V.tensor_tensor(
                        out=ykv, in0=kkv, in1=dup2(yy[:]), op=ALU.mult
                    )
                    V.tensor_tensor(
                        out=ov[:, :, s, 0], in0=xp[:], in1=ykv[:, :, 0],
                        op=ALU.add,
                    )  # x0'
                    V.tensor_tensor(
                        out=ov[:, :, s, 1], in0=x1p, in1=ykv[:, :, 1],
                        op=ALU.add,
                    )  # x1'

                    prev = dict(
                        p01=p01_n,
                        p11=p11_n[:],
                        x0=ov[:, :, s, 0],
                        x1=ov[:, :, s, 1],
                    )

                # ---- store outputs ----
                all_chunks = s0 >= W
                ov4 = o_sl[:].rearrange(
                    "p (g c x) -> p g c x", g=G, c=C, x=ns * 2
                )
                for g in range(G):
                    src = ov4[:, g] if all_chunks else ov4[:, g, 0:1, :]
                    nc.sync.dma_start(
                        out_slab_dst(g, s0, ns, all_chunks), src
                    )

    nc.compile()
    return nc


# ----------------------------------------------------------------------
# Full-problem entry point
# ----------------------------------------------------------------------
B, T = 8192, 2048
NCORES = 8
B_LOC = B // NCORES
C_CHUNKS = 16
WARM = 64
SLAB = 16

_nc_cache = {}


def _get_nc():
    key = (B_LOC, T, C_CHUNKS, WARM, SLAB)
    if key not in _nc_cache:
        _nc_cache[key] = build_core_kernel(*key)
    return _nc_cache[key]


def kernel(price: np.ndarray, hurst: np.ndarray, vol_sigma: np.ndarray) -> np.ndarray:
    from concourse import bass_utils

    price = np.ascontiguousarray(price, dtype=np.float32)
    hurst = np.ascontiguousarray(hurst, dtype=np.float32)
    vol_sigma = np.ascontiguousarray(vol_sigma, dtype=np.float32)
    nc = _get_nc()
    in_maps = []
    for k in range(NCORES):
        sl = slice(k * B_LOC, (k + 1) * B_LOC)
        in_maps.append(
            {
                "price": price[sl],
                "hurst": hurst[sl],
                "vol_sigma": vol_sigma[sl],
            }
        )
    res = bass_utils.run_bass_kernel_spmd(
        nc, in_maps, core_ids=list(range(NCORES))
    )
    return np.concatenate([r["out"] for r in res.results], axis=0)
